# revision 12
# baseline (speedup 1.0000x reference)
"""Trainium2 Bass kernel for BCGNConv (Bayesian graph conv, gnn_message_passing).

Computation (see reference):
    out_deg = hist(src); in_deg = hist(dst)
    W = weight_mu + exp(weight_logsd) * eps_w ;  kl = KL(w) + KL(b)
    h = (feat * clip(out_deg,1)^-1/2) @ W                    # [N, 64]
    rst = segment_sum(h[src], dst) * clip(in_deg,1)^-1/2 + bias

Distribution: nodes sharded 8 ways by dst ownership (graph parallel).
Every core computes the full h table with a replicated fp32r matmul
(feat pre-transposed on the host), storing h to HBM in partition-major
row order r = (n%128)*HCB + n//128 so the phase-1 writes are contiguous
per partition.

Edge phase (this runtime has no custom-GPSIMD gather/scatter ucode, so
everything is standard DMA + PE/DVE):
  per dst-block b (128 local nodes) the host groups that block's edges
  into 128-edge chunks.  For each chunk:
    - one [128,1]-offset indirect DMA gathers the 128 source h rows
      (edge p of the chunk -> partition p),
    - DVE builds a one-hot S[p, d] = (slot_p == d) from a per-chunk
      slot column (slot = dst % 128, -1 for padding),
    - PE accumulates  psum[d, f] += sum_p S[p, d] * m[p, f]
  which is exactly the segment-sum.  norm_r and bias are fused into the
  PSUM eviction.  KL and the Bayesian weight/bias sampling run on-chip.
"""

import math
import sys

import numpy as np

try:
    import concourse.bass as bass  # noqa: F401
except ImportError:  # container default location
    sys.path.insert(0, "/opt/trn_rl_repo")

import concourse.bacc as bacc
import concourse.bass as bass
import concourse.mybir as mybir
import concourse.tile as tile
from concourse.masks import make_identity

F32 = mybir.dt.float32
F32R = mybir.dt.float32r
I32 = mybir.dt.int32

N_NODES = 50000
FIN = 256
FOUT = 64
NCORES = 8
NSH = N_NODES // NCORES              # 6250 nodes per core

# h table geometry (partition-major): node n -> row (n%128)*HCB + n//128
NPAD = math.ceil(N_NODES / 128) * 128   # 50048
HCB = NPAD // 128                        # 391 columns
# output shard geometry: local node d -> (block b, slot p) = (d//128, d%128)
OSH = math.ceil(NSH / 128) * 128        # 6272
OCB = OSH // 128                         # 49 blocks


def build_program(nsub):
    """Build the SPMD Bass program. nsub[b] = number of 128-edge chunks for
    dst-block b (identical across cores; edges padded with slot=-1)."""
    total = int(sum(nsub))

    nc = bacc.Bacc()

    featT = nc.dram_tensor("featT", [FIN, NPAD], F32, kind="ExternalInput")
    gidx = nc.dram_tensor("gidx", [128, total], I32, kind="ExternalInput")
    slot = nc.dram_tensor("slot", [128, total], F32, kind="ExternalInput")
    iotat = nc.dram_tensor("iotat", [128, 128], F32, kind="ExternalInput")
    degO = nc.dram_tensor("degO", [128, HCB], F32, kind="ExternalInput")
    degI = nc.dram_tensor("degI", [128, OCB], F32, kind="ExternalInput")
    wmu = nc.dram_tensor("wmu", [FIN, FOUT], F32, kind="ExternalInput")
    wlsd = nc.dram_tensor("wlsd", [FIN, FOUT], F32, kind="ExternalInput")
    wpmu = nc.dram_tensor("wpmu", [FIN, FOUT], F32, kind="ExternalInput")
    wplsd = nc.dram_tensor("wplsd", [FIN, FOUT], F32, kind="ExternalInput")
    epsw = nc.dram_tensor("epsw", [FIN, FOUT], F32, kind="ExternalInput")
    bmu = nc.dram_tensor("bmu", [1, FOUT], F32, kind="ExternalInput")
    blsd = nc.dram_tensor("blsd", [1, FOUT], F32, kind="ExternalInput")
    bpmu = nc.dram_tensor("bpmu", [1, FOUT], F32, kind="ExternalInput")
    bplsd = nc.dram_tensor("bplsd", [1, FOUT], F32, kind="ExternalInput")
    epsb = nc.dram_tensor("epsb", [1, FOUT], F32, kind="ExternalInput")

    out = nc.dram_tensor("out", [128, OCB, FOUT], F32, kind="ExternalOutput")
    klout = nc.dram_tensor("kl", [1, 1], F32, kind="ExternalOutput")

    h = nc.dram_tensor("h", [NPAD, FOUT], F32)          # partition-major rows

    kw = FIN // 128  # K chunks (2)

    with tile.TileContext(nc) as tc:
        with (
            tc.tile_pool(name="const", bufs=1) as cp,
            tc.tile_pool(name="feat", bufs=2) as fp,
            tc.tile_pool(name="hst", bufs=2) as hp,
            tc.tile_pool(name="m", bufs=12) as mp,
            tc.tile_pool(name="sp", bufs=4) as sp,
            tc.tile_pool(name="outp", bufs=4) as op,
            tc.tile_pool(name="ps_mm", bufs=2, space="PSUM") as ps_mm,
            tc.tile_pool(name="ps_t", bufs=2, space="PSUM") as ps_t,
            tc.tile_pool(name="ps_r", bufs=2, space="PSUM") as ps_r,
            tc.tile_pool(name="ps_misc", bufs=1, space="PSUM") as ps_misc,
        ):
            # ---------------- constants / small tensors ----------------
            gidx_t = cp.tile([128, total], I32, tag="gidx")
            slot_t = cp.tile([128, total], F32, tag="slot")
            iota_t = cp.tile([128, 128], F32, tag="iota")
            nc.sync.dma_start(out=gidx_t[:], in_=gidx[:])
            nc.sync.dma_start(out=slot_t[:], in_=slot[:])
            nc.sync.dma_start(out=iota_t[:], in_=iotat[:])

            degO_t = cp.tile([128, HCB], F32, tag="degO")
            degI_t = cp.tile([128, OCB], F32, tag="degI")
            nc.sync.dma_start(out=degO_t[:], in_=degO[:])
            nc.sync.dma_start(out=degI_t[:], in_=degI[:])

            # norm_l / norm_r = 1/sqrt(max(deg,1))
            nlO = cp.tile([128, HCB], F32, tag="nlO")
            nc.vector.tensor_scalar_max(out=nlO[:], in0=degO_t[:], scalar1=1.0)
            nc.scalar.sqrt(out=nlO[:], in_=nlO[:])
            nc.vector.reciprocal(out=nlO[:], in_=nlO[:])
            nlI = cp.tile([128, OCB], F32, tag="nlI")
            nc.vector.tensor_scalar_max(out=nlI[:], in0=degI_t[:], scalar1=1.0)
            nc.scalar.sqrt(out=nlI[:], in_=nlI[:])
            nc.vector.reciprocal(out=nlI[:], in_=nlI[:])

            # weight tensors -> [128, kw, FOUT]
            def load_w(t):
                s = cp.tile([128, kw, FOUT], F32, tag=f"w_{t.name}")
                for k in range(kw):
                    nc.sync.dma_start(out=s[:, k, :], in_=t[k * 128:(k + 1) * 128, :])
                return s

            wmu_t = load_w(wmu)
            wlsd_t = load_w(wlsd)
            wpmu_t = load_w(wpmu)
            wplsd_t = load_w(wplsd)
            epsw_t = load_w(epsw)

            fw = kw * FOUT  # flattened free size (128)
            W_t = cp.tile([128, kw, FOUT], F32, tag="W")
            t1 = cp.tile([128, fw], F32, tag="t1")
            t2 = cp.tile([128, fw], F32, tag="t2")
            t3 = cp.tile([128, fw], F32, tag="t3")

            def fv(x):  # flatten [128, kw, FOUT] -> [128, fw]
                return x[:].rearrange("p k f -> p (k f)")

            # W = wmu + exp(wlsd) * epsw
            nc.scalar.activation(out=t1[:], in_=fv(wlsd_t), func=mybir.ActivationFunctionType.Exp)
            nc.vector.tensor_tensor(out=t1[:], in0=t1[:], in1=fv(epsw_t), op=mybir.AluOpType.mult)
            nc.vector.tensor_tensor(out=fv(W_t), in0=t1[:], in1=fv(wmu_t), op=mybir.AluOpType.add)

            # KL(w): sum( lp - lq + (exp(2lq) + (mq-mp)^2) * 0.5*exp(-2lp) - 0.5 )
            nc.scalar.activation(out=t1[:], in_=fv(wlsd_t), func=mybir.ActivationFunctionType.Exp, scale=2.0)
            nc.vector.tensor_tensor(out=t2[:], in0=fv(wmu_t), in1=fv(wpmu_t), op=mybir.AluOpType.subtract)
            nc.vector.tensor_tensor(out=t2[:], in0=t2[:], in1=t2[:], op=mybir.AluOpType.mult)
            nc.vector.tensor_tensor(out=t1[:], in0=t1[:], in1=t2[:], op=mybir.AluOpType.add)
            nc.scalar.activation(out=t3[:], in_=fv(wplsd_t), func=mybir.ActivationFunctionType.Exp, scale=-2.0)
            nc.vector.tensor_tensor(out=t1[:], in0=t1[:], in1=t3[:], op=mybir.AluOpType.mult)
            nc.scalar.mul(out=t1[:], in_=t1[:], mul=0.5)
            nc.vector.tensor_tensor(out=t2[:], in0=fv(wplsd_t), in1=fv(wlsd_t), op=mybir.AluOpType.subtract)
            nc.vector.tensor_tensor(out=t1[:], in0=t1[:], in1=t2[:], op=mybir.AluOpType.add)
            nc.vector.tensor_scalar_add(out=t1[:], in0=t1[:], scalar1=-0.5)
            klw_red = cp.tile([128, 1], F32, tag="klw_red")
            nc.vector.reduce_sum(out=klw_red[:], in_=t1[:], axis=mybir.AxisListType.X)

            ones_c = cp.tile([128, 1], F32, tag="ones_c")
            nc.vector.memset(ones_c[:], 1.0)
            ps_kl = ps_misc.tile([1, 1], F32, tag="ps_kl")
            nc.tensor.matmul(out=ps_kl[:], lhsT=klw_red[:], rhs=ones_c[:], start=True, stop=True)

            # bias tensors on partition 0
            def load_b(t):
                s = cp.tile([1, FOUT], F32, tag=f"b_{t.name}")
                nc.sync.dma_start(out=s[:], in_=t[:])
                return s

            bmu_t = load_b(bmu)
            blsd_t = load_b(blsd)
            bpmu_t = load_b(bpmu)
            bplsd_t = load_b(bplsd)
            epsb_t = load_b(epsb)

            b1 = cp.tile([1, FOUT], F32, tag="b1")
            b2 = cp.tile([1, FOUT], F32, tag="b2")
            b3 = cp.tile([1, FOUT], F32, tag="b3")
            bias_v = cp.tile([1, FOUT], F32, tag="bias_v")
            # bias = bmu + exp(blsd) * epsb
            nc.scalar.activation(out=b1[:], in_=blsd_t[:], func=mybir.ActivationFunctionType.Exp)
            nc.vector.tensor_tensor(out=b1[:], in0=b1[:], in1=epsb_t[:], op=mybir.AluOpType.mult)
            nc.vector.tensor_tensor(out=bias_v[:], in0=b1[:], in1=bmu_t[:], op=mybir.AluOpType.add)
            # KL(b)
            nc.scalar.activation(out=b1[:], in_=blsd_t[:], func=mybir.ActivationFunctionType.Exp, scale=2.0)
            nc.vector.tensor_tensor(out=b2[:], in0=bmu_t[:], in1=bpmu_t[:], op=mybir.AluOpType.subtract)
            nc.vector.tensor_tensor(out=b2[:], in0=b2[:], in1=b2[:], op=mybir.AluOpType.mult)
            nc.vector.tensor_tensor(out=b1[:], in0=b1[:], in1=b2[:], op=mybir.AluOpType.add)
            nc.scalar.activation(out=b3[:], in_=bplsd_t[:], func=mybir.ActivationFunctionType.Exp, scale=-2.0)
            nc.vector.tensor_tensor(out=b1[:], in0=b1[:], in1=b3[:], op=mybir.AluOpType.mult)
            nc.scalar.mul(out=b1[:], in_=b1[:], mul=0.5)
            nc.vector.tensor_tensor(out=b2[:], in0=bplsd_t[:], in1=blsd_t[:], op=mybir.AluOpType.subtract)
            nc.vector.tensor_tensor(out=b1[:], in0=b1[:], in1=b2[:], op=mybir.AluOpType.add)
            nc.vector.tensor_scalar_add(out=b1[:], in0=b1[:], scalar1=-0.5)
            klb_red = cp.tile([1, 1], F32, tag="klb_red")
            nc.vector.reduce_sum(out=klb_red[:], in_=b1[:], axis=mybir.AxisListType.X)

            kl_sb = cp.tile([1, 1], F32, tag="kl_sb")
            nc.vector.tensor_copy(out=kl_sb[:], in_=ps_kl[:])
            nc.vector.tensor_tensor(out=kl_sb[:], in0=kl_sb[:], in1=klb_red[:], op=mybir.AluOpType.add)
            nc.sync.dma_start(out=klout[:], in_=kl_sb[:])

            # broadcast bias to all 128 partitions via K=1 matmul
            ones_r = cp.tile([1, 128], F32, tag="ones_r")
            nc.vector.memset(ones_r[:], 1.0)
            ps_b = ps_misc.tile([128, FOUT], F32, tag="ps_b")
            nc.tensor.matmul(out=ps_b[:], lhsT=ones_r[:], rhs=bias_v[:], start=True, stop=True)
            bias_b = cp.tile([128, FOUT], F32, tag="bias_b")
            nc.vector.tensor_copy(out=bias_b[:], in_=ps_b[:])

            # fp32r-rounded copy of W for the main matmul
            W_r = cp.tile([128, kw, FOUT], F32R, tag="W_r")
            nc.vector.tensor_copy(out=W_r[:].rearrange("p k f -> p (k f)"), in_=fv(W_t))

            # identity for PE transpose
            ident = cp.tile([128, 128], F32, tag="ident")
            make_identity(nc, ident[:])

            # ---------------- phase 1: h = (feat * nlO) @ W ----------------
            h_v = h[:].rearrange("(p c) f -> p c f", c=HCB)
            OUTER = 2048
            for n0 in range(0, NPAD, OUTER):
                nt = min(OUTER, NPAD - n0)
                fts = []
                for k in range(kw):
                    ft = fp.tile([128, nt], F32R, tag=f"ft{k}")
                    # SWDGE cast f32 -> f32r during the load (rounds for PE)
                    nc.gpsimd.dma_start(
                        out=ft[:], in_=featT[k * 128:(k + 1) * 128, n0:n0 + nt]
                    )
                    fts.append(ft)
                hstage = hp.tile([128, nt // 128, FOUT], F32, tag="hstage")
                for s0 in range(0, nt, 512):
                    st = min(512, nt - s0)
                    ph = ps_mm.tile([FOUT, st], F32, tag="ph")
                    for k in range(kw):
                        nc.tensor.matmul(
                            out=ph[:],
                            lhsT=W_r[:, k, :],
                            rhs=fts[k][:, s0:s0 + st],
                            start=(k == 0),
                            stop=(k == kw - 1),
                        )
                    hT = hp.tile([FOUT, st], F32, tag="hT")
                    nc.vector.tensor_copy(out=hT[:], in_=ph[:])
                    for q in range(st // 128):
                        pt = ps_t.tile([128, FOUT], F32, tag="pt")
                        nc.tensor.transpose(
                            out=pt[:],
                            in_=hT[:, q * 128:(q + 1) * 128],
                            identity=ident[:FOUT, :FOUT],
                        )
                        cg = (n0 + s0 + q * 128) // 128
                        nc.vector.tensor_scalar_mul(
                            out=hstage[:, (s0 // 128) + q, :],
                            in0=pt[:],
                            scalar1=nlO[:, cg:cg + 1],
                        )
                nc.sync.dma_start(
                    out=h_v[:, n0 // 128:(n0 + nt) // 128, :], in_=hstage[:]
                )

            # ------------- phase 2: gather + one-hot segment-sum -------------
            cg = 0
            for b in range(OCB):
                pr = ps_r.tile([128, FOUT], F32, tag="pr")
                nb = nsub[b]
                for j in range(nb):
                    mt = mp.tile([128, FOUT], F32, tag="mt")
                    nc.gpsimd.indirect_dma_start(
                        out=mt[:], out_offset=None, in_=h[:],
                        in_offset=bass.IndirectOffsetOnAxis(
                            ap=gidx_t[:, cg:cg + 1], axis=0),
                    )
                    S = sp.tile([128, 128], F32, tag="S")
                    nc.vector.tensor_scalar(
                        out=S[:], in0=iota_t[:],
                        scalar1=slot_t[:, cg:cg + 1], scalar2=None,
                        op0=mybir.AluOpType.is_equal,
                    )
                    nc.tensor.matmul(
                        out=pr[:], lhsT=S[:], rhs=mt[:],
                        start=(j == 0), stop=(j == nb - 1),
                    )
                    cg += 1
                ot = op.tile([128, FOUT], F32, tag="ot")
                nc.vector.tensor_scalar_mul(out=ot[:], in0=pr[:], scalar1=nlI[:, b:b + 1])
                nc.vector.tensor_tensor(out=ot[:], in0=ot[:], in1=bias_b[:], op=mybir.AluOpType.add)
                nc.sync.dma_start(out=out[:, b, :], in_=ot[:])

    nc.finalize()
    return nc


def prepare_inputs(feat, weight_mu, weight_logsd, weight_prior_mu,
                   weight_prior_logsd, bias_mu, bias_logsd, bias_prior_mu,
                   bias_prior_logsd, eps_w, eps_b, src, dst):
    """Host-side sharding/layout prep. Returns (in_maps, nsub)."""
    src = np.asarray(src).astype(np.int64)
    dst = np.asarray(dst).astype(np.int64)
    feat = np.asarray(feat, dtype=np.float32)

    featT = np.zeros((FIN, NPAD), dtype=np.float32)
    featT[:, :N_NODES] = feat.T
    featT = np.ascontiguousarray(featT)

    out_deg = np.bincount(src, minlength=N_NODES).astype(np.float32)
    in_deg = np.bincount(dst, minlength=N_NODES).astype(np.float32)
    degO = np.pad(out_deg, (0, NPAD - N_NODES), constant_values=1.0)
    degO = np.ascontiguousarray(degO.reshape(HCB, 128).T)

    iota_t = np.ascontiguousarray(
        np.broadcast_to(np.arange(128, dtype=np.float32), (128, 128)))

    # h-table row remap (partition-major)
    r_src_all = ((src % 128) * HCB + src // 128).astype(np.int64)

    owner = dst // NSH
    # per (core, block) edge lists
    per_core = []
    counts = np.zeros((NCORES, OCB), dtype=np.int64)
    for i in range(NCORES):
        sel = np.nonzero(owner == i)[0]
        rs = r_src_all[sel]
        dl = dst[sel] - i * NSH
        blk = dl // 128
        slt = dl % 128
        order = np.argsort(blk, kind="stable")
        rs, blk, slt = rs[order], blk[order], slt[order]
        bnd = np.searchsorted(blk, np.arange(OCB + 1))
        per_core.append((rs, slt, bnd))
        counts[i] = bnd[1:] - bnd[:-1]

    # uniform per-block chunk counts (max over cores, rounded to 128)
    bmax = counts.max(axis=0)
    nsub = [max(1, int(math.ceil(m / 128))) for m in bmax]
    total = int(sum(nsub))

    wf = np.float32
    shared = dict(
        featT=featT,
        degO=degO,
        iotat=iota_t,
        wmu=np.ascontiguousarray(weight_mu, dtype=wf),
        wlsd=np.ascontiguousarray(weight_logsd, dtype=wf),
        wpmu=np.ascontiguousarray(weight_prior_mu, dtype=wf),
        wplsd=np.ascontiguousarray(weight_prior_logsd, dtype=wf),
        epsw=np.ascontiguousarray(eps_w, dtype=wf),
        bmu=np.ascontiguousarray(bias_mu, dtype=wf),
        blsd=np.ascontiguousarray(bias_logsd, dtype=wf),
        bpmu=np.ascontiguousarray(bias_prior_mu, dtype=wf),
        bplsd=np.ascontiguousarray(bias_prior_logsd, dtype=wf),
        epsb=np.ascontiguousarray(eps_b, dtype=wf),
    )

    in_maps = []
    for i in range(NCORES):
        rs, slt, bnd = per_core[i]
        gidx = np.zeros((128, total), dtype=np.int32)
        slot = np.full((128, total), -1.0, dtype=np.float32)
        cg = 0
        for b in range(OCB):
            e0, e1 = bnd[b], bnd[b + 1]
            n = e1 - e0
            cap = nsub[b] * 128
            g = np.zeros(cap, dtype=np.int32)
            s = np.full(cap, -1.0, dtype=np.float32)
            g[:n] = rs[e0:e1]
            s[:n] = slt[e0:e1].astype(np.float32)
            gidx[:, cg:cg + nsub[b]] = g.reshape(nsub[b], 128).T
            slot[:, cg:cg + nsub[b]] = s.reshape(nsub[b], 128).T
            cg += nsub[b]

        shard = in_deg[i * NSH:(i + 1) * NSH]
        degI = np.pad(shard, (0, OSH - NSH), constant_values=1.0)
        degI = np.ascontiguousarray(degI.reshape(OCB, 128).T)

        m = dict(shared)
        m["gidx"] = gidx
        m["slot"] = slot
        m["degI"] = degI
        in_maps.append(m)

    return in_maps, nsub


def assemble_output(results):
    rst = np.empty((N_NODES, FOUT), dtype=np.float32)
    for i in range(NCORES):
        o = results[i]["out"]  # [128, OCB, FOUT]; [p, b] = local node b*128+p
        shard = o.transpose(1, 0, 2).reshape(OSH, FOUT)[:NSH]
        rst[i * NSH:(i + 1) * NSH] = shard
    kl = np.float32(results[0]["kl"][0, 0])
    return rst, kl


_RUN_KW = {}


def kernel(**inputs):
    from concourse.bass_utils import run_bass_kernel_spmd

    in_maps, nsub = prepare_inputs(**inputs)
    nc = build_program(nsub)
    res = run_bass_kernel_spmd(nc, in_maps, core_ids=list(range(NCORES)), **_RUN_KW)
    if _RUN_KW.get("trace"):
        kernel.last_results = res
    return assemble_output(res.results)
